# revision 7
# baseline (speedup 1.0000x reference)
"""Grouped projected head on 8 TRN2 NeuronCores — v2.

Sharding: group axis G=16 split across 8 cores (2 groups/core, expert-parallel).
z replicated (pre-transposed + bf16 on host). Per core, for its groups g:
    h = z @ W1[g] + b1[g] -> LayerNorm -> GELU(exact)
    q = h @ W2[g] + b2[g] -> L2 normalize -> * min(exp(ls[g]),100)
    logits = q @ normalize(Wv[g]).T + bv[g]

v2 changes vs baseline:
  - No ACT table thrash: every ACT func in steady state is in the
    gelu_and_others set (Gelu, Square, Copy) -> one table load total.
    sqrt/rsqrt done on DVE via bit-trick seed + one Newton step; the result
    is produced NEGATED and the sign is absorbed into host-negated
    constants (-ln_g for the LN scale, -min(exp(ls),100) for the q scale)
    or an extra *-1 ALU stage (Wv rows).
  - LN stats: h-evacuation is scalar_tensor_tensor with accum_out (sum h
    for free); sum h^2 via ACT Square+accum (idle engine) -> bn_stats and
    its DVE cost removed.
  - PSUM->SBUF transpose-evacuation copies moved from DVE to ACT.
  - min(exp(logit_scale),100) precomputed on host.
  - Optional bf16 DRAM output (halves output HBM traffic), upcast on host.
"""

import sys

sys.path.insert(0, "/opt/trn_rl_repo")

import numpy as np
import ml_dtypes

import concourse.bass as bass
from concourse import bacc, mybir, tile

BF16 = mybir.dt.bfloat16
F32 = mybir.dt.float32
I32 = mybir.dt.int32
AF = mybir.ActivationFunctionType
ALU = mybir.AluOpType

B, G, IN, HID, PROJ, CHUNK = 4096, 16, 1024, 2048, 256, 4096
NCORES = 8
GPC = G // NCORES          # groups per core
NB = B // 128              # 32 batch blocks
KI = IN // 128             # 8 k-chunks for mm1
TH = HID // 128            # 16 hid-chunks
NVB = CHUNK // 128         # 32 Wv row blocks
LN_EPS = 1e-5
MAGIC = 0x5EF759DF         # rsqrt seed magic for half-x input
GELU_FUNC = None           # set to AF.Tanh for sim debug
OUT_BF16 = True
OUT_DT = BF16 if OUT_BF16 else F32

_RT = None  # cached (nc, put, run)


def _bc(ap, parts=128):
    """Partition-broadcast a DRAM AP (stride-0 partition dim) for DMA."""
    return bass.AP(tensor=ap.tensor, offset=ap.offset, ap=[[0, parts], *ap.ap])


def _build():
    nc = bacc.Bacc("TRN2", target_bir_lowering=False, debug=False)

    zt_h = nc.dram_tensor("zt", [NB, 128, KI, 128], BF16, kind="ExternalInput")
    w1_h = nc.dram_tensor("w1", [GPC, 128, KI, HID], BF16, kind="ExternalInput")
    w2_h = nc.dram_tensor("w2", [GPC, 128, TH, PROJ], BF16, kind="ExternalInput")
    wv_h = nc.dram_tensor("wv", [GPC, CHUNK, PROJ], BF16, kind="ExternalInput")
    b1_h = nc.dram_tensor("b1", [GPC, HID], BF16, kind="ExternalInput")
    b2_h = nc.dram_tensor("b2", [GPC, PROJ], F32, kind="ExternalInput")
    bv_h = nc.dram_tensor("bv", [GPC, CHUNK], BF16, kind="ExternalInput")
    lngn_h = nc.dram_tensor("lngn", [128, GPC, TH], F32, kind="ExternalInput")
    lnb_h = nc.dram_tensor("lnb", [128, GPC, TH], F32, kind="ExternalInput")
    sneg_h = nc.dram_tensor("sneg", [GPC], F32, kind="ExternalInput")
    out_h = nc.dram_tensor("out", [B, GPC * CHUNK], OUT_DT, kind="ExternalOutput")

    with tile.TileContext(nc) as tc:
        with (
            tc.tile_pool(name="consts", bufs=1) as consts,
            tc.tile_pool(name="w1p", bufs=2) as w1p,
            tc.tile_pool(name="wtp", bufs=2) as wtp,
            tc.tile_pool(name="ztp", bufs=4) as ztp,
            tc.tile_pool(name="hp", bufs=2) as hp,
            tc.tile_pool(name="htp", bufs=2) as htp,
            tc.tile_pool(name="small", bufs=3) as small,
            tc.tile_pool(name="tiny", bufs=4) as tiny,
            tc.tile_pool(name="wvraw", bufs=2) as wvraw,
            tc.tile_pool(name="wvs", bufs=2) as wvs,
            tc.tile_pool(name="lop", bufs=3) as lop,
            tc.tile_pool(name="psA", bufs=2, space="PSUM") as psA,
            tc.tile_pool(name="psT", bufs=3, space="PSUM") as psT,
            tc.tile_pool(name="psQ", bufs=1, space="PSUM") as psQ,
            tc.tile_pool(name="psL", bufs=2, space="PSUM") as psL,
        ):
            from concourse.masks import make_identity

            def rsqrt_neg(x_ap, n, eps, pool=tiny):
                """Emit DVE ops computing ~ -1/sqrt(x+eps) for [128, n] fp32.

                Bit-trick seed + one Newton step; returns the tile holding the
                NEGATED result (|rel err| <= ~1.8e-3)."""
                xh = pool.tile([128, n], F32, tag="rs_xh")
                nc.vector.tensor_scalar(out=xh[:], in0=x_ap, scalar1=eps,
                                        scalar2=0.5, op0=ALU.add, op1=ALU.mult)
                y0 = pool.tile([128, n], F32, tag="rs_y0")
                nc.vector.tensor_scalar(out=y0[:].bitcast(I32),
                                        in0=xh[:].bitcast(I32), scalar1=1,
                                        scalar2=None, op0=ALU.arith_shift_right)
                nc.vector.tensor_scalar(out=y0[:].bitcast(I32),
                                        in0=y0[:].bitcast(I32), scalar1=-1,
                                        scalar2=MAGIC, op0=ALU.mult, op1=ALU.add)
                a = pool.tile([128, n], F32, tag="rs_a")
                nc.vector.tensor_tensor(out=a[:], in0=y0[:], in1=y0[:], op=ALU.mult)
                nc.vector.tensor_tensor(out=a[:], in0=a[:], in1=xh[:], op=ALU.mult)
                z = pool.tile([128, n], F32, tag="rs_z")
                nc.vector.scalar_tensor_tensor(
                    out=z[:], in0=a[:], scalar=1.5, in1=y0[:],
                    op0=ALU.subtract, op1=ALU.mult,
                )  # (xh*y0^2 - 1.5)*y0 = -y1
                return z, xh

            def newton_again(z, xh, n, pool=tiny):
                """One more Newton step; input negated -> output positive."""
                a = pool.tile([128, n], F32, tag="rs_a2")
                nc.vector.tensor_tensor(out=a[:], in0=z[:], in1=z[:], op=ALU.mult)
                nc.vector.tensor_tensor(out=a[:], in0=a[:], in1=xh[:], op=ALU.mult)
                y = pool.tile([128, n], F32, tag="rs_y2")
                nc.vector.scalar_tensor_tensor(
                    out=y[:], in0=a[:], scalar=1.5, in1=z[:],
                    op0=ALU.subtract, op1=ALU.mult,
                )
                return y

            def wv_dma(gl, vh):
                wvbig = wvraw.tile([128, 16, PROJ], BF16, tag="wvbig")
                nc.gpsimd.dma_start(
                    out=wvbig[:],
                    in_=wv_h.ap()[gl, vh * 2048 : (vh + 1) * 2048, :].rearrange(
                        "(a p) n -> p a n", p=128
                    ),
                )
                return wvbig

            # ---------------- startup: critical-path DMAs first ----------------
            # mm3 of block 0 first needs wv half-1 (columns 0..2047); mm1 needs
            # w1[0] + zt[0] + b1. Issue those before the rest of the constants.
            wv_first = wv_dma(0, 0)
            w1_sbs = {}
            w1_first = w1p.tile([128, KI, HID], BF16, tag="w1")
            w1_sbs[0] = w1_first
            nc.gpsimd.dma_start(out=w1_first[:], in_=w1_h.ap()[0])
            zt0 = ztp.tile([128, KI, 128], BF16, tag="zt")
            nc.sync.dma_start(out=zt0[:], in_=zt_h.ap()[0])
            b1_rep = consts.tile([128, GPC, HID], BF16)
            nc.gpsimd.dma_start(out=b1_rep[:], in_=_bc(b1_h.ap()))
            wv_second = wv_dma(0, 1)

            ident = consts.tile([128, 128], BF16)
            make_identity(nc, ident[:])

            b2_rep = consts.tile([128, GPC, PROJ], F32)
            nc.gpsimd.dma_start(out=b2_rep[:], in_=_bc(b2_h.ap()))
            bv_rep = consts.tile([128, GPC, CHUNK], BF16)
            nc.gpsimd.dma_start(out=bv_rep[:], in_=_bc(bv_h.ap()))
            lngn_sb = consts.tile([128, GPC, TH], F32)
            nc.gpsimd.dma_start(out=lngn_sb[:], in_=lngn_h.ap())
            lnb_sb = consts.tile([128, GPC, TH], F32)
            nc.gpsimd.dma_start(out=lnb_sb[:], in_=lnb_h.ap())
            w2_sb = consts.tile([128, GPC, TH, PROJ], BF16)
            nc.gpsimd.dma_start(out=w2_sb[:], in_=w2_h.ap().rearrange("g p t n -> p g t n"))
            sneg_sb = consts.tile([128, GPC], F32)
            nc.gpsimd.dma_start(out=sneg_sb[:], in_=_bc(sneg_h.ap()))

            def w1_load(gl):
                t = w1p.tile([128, KI, HID], BF16, tag="w1")
                nc.gpsimd.dma_start(out=t[:], in_=w1_h.ap()[gl])
                return t

            def wv_prep_half(wT, wvbig, vh):
                """Row-normalize + transpose one half of Wv into wT columns."""
                wss = wvs.tile([128, 16], F32, tag="wss")
                for i in range(16):
                    wjunk = wvs.tile([128, PROJ], BF16, tag="wjunk")
                    nc.scalar.activation(
                        out=wjunk[:], in_=wvbig[:, i, :], func=AF.Square,
                        accum_out=wss[:, i : i + 1],
                    )
                zneg, wxh = rsqrt_neg(wss[:], 16, 1e-24)
                rw = newton_again(zneg, wxh, 16)  # positive rsqrt, 2 Newtons
                for i in range(16):
                    vb = vh * 16 + i
                    wn = wvs.tile([128, PROJ], BF16, tag="wn")
                    nc.vector.tensor_scalar_mul(
                        out=wn[:], in0=wvbig[:, i, :], scalar1=rw[:, i : i + 1]
                    )
                    for j in range(2):
                        ptw = psT.tile([128, 128], BF16, tag="pt")
                        nc.tensor.transpose(
                            out=ptw[:], in_=wn[:, j * 128 : (j + 1) * 128],
                            identity=ident[:],
                        )
                        nc.scalar.activation(
                            out=wT[:, j, vb * 128 : (vb + 1) * 128], in_=ptw[:],
                            func=AF.Copy,
                        )

            def wv_prep(gl, halves=None):
                """Wv row-normalize + transpose -> wT [128, 2, CHUNK]."""
                wT = wtp.tile([128, 2, CHUNK], BF16, tag="wT")
                if halves is None:
                    halves = [wv_dma(gl, 0), wv_dma(gl, 1)]
                for vh in range(2):
                    wv_prep_half(wT, halves[vh], vh)
                return wT

            wTs = {0: wv_prep(0, halves=[wv_first, wv_second])}
            wv_pend = {}

            for gl in range(GPC):
                w1_sb = w1_sbs[gl]
                wT = wTs[gl]

                # ---------------- main loop over batch blocks ----------------
                for bb in range(NB):
                    # software-pipeline next group's weight loads + Wv prep
                    # into the tail of this group's block loop, spread out to
                    # avoid DMA/ACT bursts
                    if gl + 1 < GPC and bb == NB - 16:
                        w1_sbs[gl + 1] = w1_load(gl + 1)
                    if gl + 1 < GPC and bb == NB - 12:
                        wv_pend[0] = wv_dma(gl + 1, 0)
                        nwT = wtp.tile([128, 2, CHUNK], BF16, tag="wT")
                        wTs[gl + 1] = nwT
                    if gl + 1 < GPC and bb == NB - 10:
                        wv_prep_half(wTs[gl + 1], wv_pend[0], 0)
                        wv_pend[1] = wv_dma(gl + 1, 1)
                    if gl + 1 < GPC and bb == NB - 5:
                        wv_prep_half(wTs[gl + 1], wv_pend[1], 1)
                    if gl == 0 and bb == 0:
                        zt_t = zt0
                    else:
                        zt_t = ztp.tile([128, KI, 128], BF16, tag="zt")
                        nc.sync.dma_start(out=zt_t[:], in_=zt_h.ap()[bb])

                    # mm1: h = z @ W1 (+b1), into 4 psum tiles of [128, 512]
                    h_sb = hp.tile([128, HID], BF16)
                    hsum = small.tile([128, 4], F32, tag="hsum")
                    hsq = small.tile([128, 4], F32, tag="hsq")
                    for nt in range(4):
                        ph = psA.tile([128, 512], F32)
                        for k in range(KI):
                            nc.tensor.matmul(
                                ph[:], zt_t[:, k, :],
                                w1_sb[:, k, nt * 512 : (nt + 1) * 512],
                                start=(k == 0), stop=(k == KI - 1),
                            )
                        hs = h_sb[:, nt * 512 : (nt + 1) * 512]
                        nc.vector.scalar_tensor_tensor(
                            out=hs, in0=ph[:], scalar=0.0,
                            in1=b1_rep[:, gl, nt * 512 : (nt + 1) * 512],
                            op0=ALU.add, op1=ALU.add,
                            accum_out=hsum[:, nt : nt + 1],
                        )
                        hjunk = small.tile([128, 512], BF16, tag="hjunk")
                        nc.scalar.activation(
                            out=hjunk[:], in_=hs, func=AF.Square,
                            accum_out=hsq[:, nt : nt + 1],
                        )

                    # mean/var from accumulated sums, -rstd via DVE rsqrt
                    hsumt = tiny.tile([128, 1], F32, tag="hsumt")
                    nc.vector.reduce_sum(hsumt[:], hsum[:], axis=mybir.AxisListType.X)
                    hsqt = tiny.tile([128, 1], F32, tag="hsqt")
                    nc.vector.reduce_sum(hsqt[:], hsq[:], axis=mybir.AxisListType.X)
                    mean = tiny.tile([128, 1], F32, tag="mean")
                    nc.vector.tensor_scalar_mul(out=mean[:], in0=hsumt[:],
                                                scalar1=1.0 / HID)
                    msq = tiny.tile([128, 1], F32, tag="msq")
                    nc.vector.tensor_tensor(out=msq[:], in0=mean[:], in1=mean[:],
                                            op=ALU.mult)
                    # hmadj = 0.5*msq - 0.5*eps
                    nc.vector.tensor_scalar(out=msq[:], in0=msq[:], scalar1=0.5,
                                            scalar2=0.5 * LN_EPS, op0=ALU.mult,
                                            op1=ALU.subtract)
                    # xh = (var+eps)/2 = hsqt*(0.5/HID) - hmadj
                    vxh = tiny.tile([128, 1], F32, tag="vxh")
                    nc.vector.scalar_tensor_tensor(
                        out=vxh[:], in0=hsqt[:], scalar=0.5 / HID, in1=msq[:],
                        op0=ALU.mult, op1=ALU.subtract,
                    )
                    # seed + one Newton (negated result)
                    y0 = tiny.tile([128, 1], F32, tag="ln_y0")
                    nc.vector.tensor_scalar(out=y0[:].bitcast(I32),
                                            in0=vxh[:].bitcast(I32), scalar1=1,
                                            scalar2=None, op0=ALU.arith_shift_right)
                    nc.vector.tensor_scalar(out=y0[:].bitcast(I32),
                                            in0=y0[:].bitcast(I32), scalar1=-1,
                                            scalar2=MAGIC, op0=ALU.mult, op1=ALU.add)
                    aa = tiny.tile([128, 1], F32, tag="ln_a")
                    nc.vector.tensor_tensor(out=aa[:], in0=y0[:], in1=y0[:], op=ALU.mult)
                    nc.vector.tensor_scalar(out=aa[:], in0=aa[:], scalar1=vxh[:],
                                            scalar2=1.5, op0=ALU.mult, op1=ALU.subtract)
                    nrstd = tiny.tile([128, 1], F32, tag="nrstd")
                    nc.vector.tensor_tensor(out=nrstd[:], in0=aa[:], in1=y0[:],
                                            op=ALU.mult)
                    # h = (h - mu) * (-rstd)  [negated; fixed by -ln_g ACT scale]
                    nc.vector.tensor_scalar(
                        out=h_sb[:], in0=h_sb[:], scalar1=mean[:], scalar2=nrstd[:],
                        op0=ALU.subtract, op1=ALU.mult,
                    )

                    # transpose + fused LN-affine (-g) + exact GELU
                    hT = htp.tile([128, TH, 128], BF16)
                    for t in range(TH):
                        pt = psT.tile([128, 128], BF16, tag="pt")
                        nc.tensor.transpose(
                            out=pt[:], in_=h_sb[:, t * 128 : (t + 1) * 128],
                            identity=ident[:],
                        )
                        nc.scalar.activation(
                            out=hT[:, t, :], in_=pt[:], func=(GELU_FUNC or AF.Gelu),
                            scale=lngn_sb[:, gl, t : t + 1],
                            bias=lnb_sb[:, gl, t : t + 1],
                        )

                    # mm2: q = h @ W2
                    pq = psQ.tile([128, PROJ], F32)
                    for t in range(TH):
                        nc.tensor.matmul(
                            pq[:], hT[:, t, :], w2_sb[:, gl, t, :],
                            start=(t == 0), stop=(t == TH - 1),
                        )
                    q_sb = small.tile([128, PROJ], F32, tag="q_sb")
                    nc.vector.tensor_tensor(
                        out=q_sb[:], in0=pq[:], in1=b2_rep[:, gl, :], op=ALU.add
                    )
                    qjunk = small.tile([128, PROJ], BF16, tag="qjunk")
                    qss = tiny.tile([128, 1], F32, tag="qss")
                    nc.scalar.activation(
                        out=qjunk[:], in_=q_sb[:], func=AF.Square, accum_out=qss[:],
                    )
                    nrq, _ = rsqrt_neg(qss[:], 1, 1e-24)
                    qsc = tiny.tile([128, 1], F32, tag="qsc")
                    nc.vector.tensor_tensor(
                        out=qsc[:], in0=nrq[:], in1=sneg_sb[:, gl : gl + 1], op=ALU.mult
                    )
                    qn = small.tile([128, PROJ], BF16, tag="qn")
                    nc.vector.tensor_scalar_mul(out=qn[:], in0=q_sb[:], scalar1=qsc[:])
                    qT = small.tile([128, 2, 128], BF16, tag="qT")
                    for j in range(2):
                        ptq = psT.tile([128, 128], BF16, tag="pt")
                        nc.tensor.transpose(
                            out=ptq[:], in_=qn[:, j * 128 : (j + 1) * 128],
                            identity=ident[:],
                        )
                        nc.scalar.activation(out=qT[:, j, :], in_=ptq[:], func=AF.Copy)

                    # mm3: logits = q @ wT (+bv), 8 tiles of 512
                    for vh in range(2):
                        lo = lop.tile([128, 4, 512], OUT_DT)
                        for v4 in range(4):
                            vt = vh * 4 + v4
                            pl = psL.tile([128, 512], F32)
                            nc.tensor.matmul(
                                pl[:], qT[:, 0, :],
                                wT[:, 0, vt * 512 : (vt + 1) * 512],
                                start=True, stop=False,
                            )
                            nc.tensor.matmul(
                                pl[:], qT[:, 1, :],
                                wT[:, 1, vt * 512 : (vt + 1) * 512],
                                start=False, stop=True,
                            )
                            nc.vector.tensor_tensor(
                                out=lo[:, v4, :], in0=pl[:],
                                in1=bv_rep[:, gl, vt * 512 : (vt + 1) * 512],
                                op=ALU.add,
                            )
                        nc.sync.dma_start(
                            out=out_h.ap()[
                                bb * 128 : (bb + 1) * 128,
                                gl * CHUNK + vh * 2048 : gl * CHUNK + (vh + 1) * 2048,
                            ],
                            in_=lo[:].rearrange("p a b -> p (a b)"),
                        )

    nc.compile()
    return nc


def _make_runner(nc):
    """Reusable jitted SPMD executor (mirrors bass2jax.run_bass_via_pjrt)."""
    import jax
    from jax.sharding import Mesh, PartitionSpec, NamedSharding
    from jax.experimental.shard_map import shard_map
    from concourse.bass2jax import _bass_exec_p, partition_id_tensor, install_neuronx_cc_hook

    install_neuronx_cc_hook()
    partition_name = nc.partition_id_tensor.name if nc.partition_id_tensor else None
    in_names, out_names, out_avals = [], [], []
    for alloc in nc.m.functions[0].allocations:
        if not isinstance(alloc, mybir.MemoryLocationSet):
            continue
        name = alloc.memorylocations[0].name
        if alloc.kind == "ExternalInput":
            if name != partition_name:
                in_names.append(name)
        elif alloc.kind == "ExternalOutput":
            out_names.append(name)
            out_avals.append(
                jax.core.ShapedArray(tuple(alloc.tensor_shape), mybir.dt.np(alloc.dtype))
            )
    n_params = len(in_names)
    all_in_names = in_names + out_names
    if partition_name is not None:
        all_in_names.append(partition_name)

    def _body(*args):
        operands = list(args)
        if partition_name is not None:
            operands.append(partition_id_tensor())
        return tuple(
            _bass_exec_p.bind(
                *operands,
                out_avals=tuple(out_avals),
                in_names=tuple(all_in_names),
                out_names=tuple(out_names),
                lowering_input_output_aliases=(),
                sim_require_finite=True,
                sim_require_nnan=True,
                nc=nc,
            )
        )

    devices = jax.devices()[:NCORES]
    mesh = Mesh(np.asarray(devices), ("core",))
    spec = NamedSharding(mesh, PartitionSpec("core"))
    n_out = len(out_names)
    fn = jax.jit(
        shard_map(
            _body, mesh=mesh,
            in_specs=(PartitionSpec("core"),) * (n_params + n_out),
            out_specs=(PartitionSpec("core"),) * n_out,
            check_rep=False,
        ),
        keep_unused=True,
    )

    def put(in_maps):
        import jax as _jax
        concat = [
            _jax.device_put(
                np.concatenate([np.asarray(in_maps[c][nm]) for c in range(NCORES)], axis=0),
                spec,
            )
            for nm in in_names
        ]
        zeros = [
            _jax.device_put(
                np.zeros((NCORES * a.shape[0], *a.shape[1:]), a.dtype), spec
            )
            for a in out_avals
        ]
        return concat + zeros

    def run(args):
        outs = fn(*args)
        return outs, out_names, out_avals

    return put, run


def _prep_inputs(z, W1, b1, ln_g, ln_b, W2, b2, Wv, bv, logit_scale):
    bf = ml_dtypes.bfloat16
    zt = np.ascontiguousarray(
        z.T.reshape(KI, 128, NB, 128).transpose(2, 1, 0, 3)
    ).astype(bf)  # [bb, p, k, b]
    s = np.minimum(np.exp(logit_scale.astype(np.float64)), 100.0).astype(np.float32)
    in_maps = []
    for c in range(NCORES):
        gs = slice(GPC * c, GPC * (c + 1))
        w1c = np.ascontiguousarray(
            W1[gs].reshape(GPC, KI, 128, HID).transpose(0, 2, 1, 3)
        ).astype(bf)  # [g, p, k, n]
        w2c = np.ascontiguousarray(
            W2[gs].reshape(GPC, TH, 128, PROJ).transpose(0, 2, 1, 3)
        ).astype(bf)  # [g, p, t, n]
        lngnc = np.ascontiguousarray(
            (-ln_g[gs]).reshape(GPC, TH, 128).transpose(2, 0, 1)
        ).astype(np.float32)  # [p, g, t], negated
        lnbc = np.ascontiguousarray(
            ln_b[gs].reshape(GPC, TH, 128).transpose(2, 0, 1)
        ).astype(np.float32)
        in_maps.append(
            {
                "zt": zt,
                "w1": w1c,
                "w2": w2c,
                "wv": Wv[gs].astype(bf),
                "b1": b1[gs].astype(bf),
                "b2": b2[gs].astype(np.float32),
                "bv": bv[gs].astype(bf),
                "lngn": lngnc,
                "lnb": lnbc,
                "sneg": -s[gs],
            }
        )
    return in_maps


def _get_runtime():
    global _RT
    if _RT is None:
        nc = _build()
        put, run = _make_runner(nc)
        _RT = (nc, put, run)
    return _RT


def kernel(**inputs):
    inputs = {k: np.asarray(v) for k, v in inputs.items()}
    in_maps = _prep_inputs(**inputs)
    _, put, run = _get_runtime()
    args = put(in_maps)
    outs, out_names, out_avals = run(args)
    out = np.asarray(outs[out_names.index("out")])
    out = out.reshape(NCORES, B, GPC * CHUNK)
    return np.concatenate(list(out), axis=1).astype(np.float32)


# revision 8
# speedup vs baseline: 1.1241x; 1.1241x over previous
"""Grouped projected head on 8 TRN2 NeuronCores — v2.

Sharding: group axis G=16 split across 8 cores (2 groups/core, expert-parallel).
z replicated (pre-transposed + bf16 on host). Per core, for its groups g:
    h = z @ W1[g] + b1[g] -> LayerNorm -> GELU(exact)
    q = h @ W2[g] + b2[g] -> L2 normalize -> * min(exp(ls[g]),100)
    logits = q @ normalize(Wv[g]).T + bv[g]

v2 changes vs baseline:
  - No ACT table thrash: every ACT func in steady state is in the
    gelu_and_others set (Gelu, Square, Copy) -> one table load total.
    sqrt/rsqrt done on DVE via bit-trick seed + one Newton step; the result
    is produced NEGATED and the sign is absorbed into host-negated
    constants (-ln_g for the LN scale, -min(exp(ls),100) for the q scale)
    or an extra *-1 ALU stage (Wv rows).
  - LN stats: h-evacuation is scalar_tensor_tensor with accum_out (sum h
    for free); sum h^2 via ACT Square+accum (idle engine) -> bn_stats and
    its DVE cost removed.
  - PSUM->SBUF transpose-evacuation copies moved from DVE to ACT.
  - min(exp(logit_scale),100) precomputed on host.
  - Optional bf16 DRAM output (halves output HBM traffic), upcast on host.
"""

import sys

sys.path.insert(0, "/opt/trn_rl_repo")

import numpy as np
import ml_dtypes

import concourse.bass as bass
from concourse import bacc, mybir, tile

BF16 = mybir.dt.bfloat16
F32 = mybir.dt.float32
I32 = mybir.dt.int32
AF = mybir.ActivationFunctionType
ALU = mybir.AluOpType

B, G, IN, HID, PROJ, CHUNK = 4096, 16, 1024, 2048, 256, 4096
NCORES = 8
GPC = G // NCORES          # groups per core
NB = B // 128              # 32 batch blocks
KI = IN // 128             # 8 k-chunks for mm1
TH = HID // 128            # 16 hid-chunks
NVB = CHUNK // 128         # 32 Wv row blocks
LN_EPS = 1e-5
MAGIC = 0x5EF759DF         # rsqrt seed magic for half-x input
GELU_FUNC = None           # set to AF.Tanh for sim debug
OUT_BF16 = True
OUT_DT = BF16 if OUT_BF16 else F32

_RT = None  # cached (nc, put, run)


def _bc(ap, parts=128):
    """Partition-broadcast a DRAM AP (stride-0 partition dim) for DMA."""
    return bass.AP(tensor=ap.tensor, offset=ap.offset, ap=[[0, parts], *ap.ap])


def _build():
    nc = bacc.Bacc("TRN2", target_bir_lowering=False, debug=False)

    zt_h = nc.dram_tensor("zt", [NB, 128, KI, 128], BF16, kind="ExternalInput")
    w1_h = nc.dram_tensor("w1", [GPC, 128, KI, HID], BF16, kind="ExternalInput")
    w2_h = nc.dram_tensor("w2", [GPC, 128, TH, PROJ], BF16, kind="ExternalInput")
    wv_h = nc.dram_tensor("wv", [GPC, CHUNK, PROJ], BF16, kind="ExternalInput")
    b1_h = nc.dram_tensor("b1", [GPC, HID], BF16, kind="ExternalInput")
    b2_h = nc.dram_tensor("b2", [GPC, PROJ], F32, kind="ExternalInput")
    bv_h = nc.dram_tensor("bv", [GPC, CHUNK], BF16, kind="ExternalInput")
    lngn_h = nc.dram_tensor("lngn", [128, GPC, TH], F32, kind="ExternalInput")
    lnb_h = nc.dram_tensor("lnb", [128, GPC, TH], F32, kind="ExternalInput")
    sneg_h = nc.dram_tensor("sneg", [GPC], F32, kind="ExternalInput")
    out_h = nc.dram_tensor("out", [B, GPC * CHUNK], OUT_DT, kind="ExternalOutput")

    with tile.TileContext(nc) as tc:
        with (
            tc.tile_pool(name="consts", bufs=1) as consts,
            tc.tile_pool(name="w1p", bufs=2) as w1p,
            tc.tile_pool(name="wtp", bufs=2) as wtp,
            tc.tile_pool(name="ztp", bufs=4) as ztp,
            tc.tile_pool(name="hp", bufs=2) as hp,
            tc.tile_pool(name="htp", bufs=2) as htp,
            tc.tile_pool(name="small", bufs=3) as small,
            tc.tile_pool(name="tiny", bufs=4) as tiny,
            tc.tile_pool(name="wvraw", bufs=2) as wvraw,
            tc.tile_pool(name="wvs", bufs=2) as wvs,
            tc.tile_pool(name="lop", bufs=3) as lop,
            tc.tile_pool(name="psA", bufs=2, space="PSUM") as psA,
            tc.tile_pool(name="psT", bufs=3, space="PSUM") as psT,
            tc.tile_pool(name="psQ", bufs=1, space="PSUM") as psQ,
            tc.tile_pool(name="psL", bufs=2, space="PSUM") as psL,
        ):
            from concourse.masks import make_identity

            def rsqrt_neg(x_ap, n, eps, pool=tiny):
                """Emit DVE ops computing ~ -1/sqrt(x+eps) for [128, n] fp32.

                Bit-trick seed + one Newton step; returns the tile holding the
                NEGATED result (|rel err| <= ~1.8e-3)."""
                xh = pool.tile([128, n], F32, tag="rs_xh")
                nc.vector.tensor_scalar(out=xh[:], in0=x_ap, scalar1=eps,
                                        scalar2=0.5, op0=ALU.add, op1=ALU.mult)
                y0 = pool.tile([128, n], F32, tag="rs_y0")
                nc.vector.tensor_scalar(out=y0[:].bitcast(I32),
                                        in0=xh[:].bitcast(I32), scalar1=1,
                                        scalar2=None, op0=ALU.arith_shift_right)
                nc.vector.tensor_scalar(out=y0[:].bitcast(I32),
                                        in0=y0[:].bitcast(I32), scalar1=-1,
                                        scalar2=MAGIC, op0=ALU.mult, op1=ALU.add)
                a = pool.tile([128, n], F32, tag="rs_a")
                nc.vector.tensor_tensor(out=a[:], in0=y0[:], in1=y0[:], op=ALU.mult)
                nc.vector.tensor_tensor(out=a[:], in0=a[:], in1=xh[:], op=ALU.mult)
                z = pool.tile([128, n], F32, tag="rs_z")
                nc.vector.scalar_tensor_tensor(
                    out=z[:], in0=a[:], scalar=1.5, in1=y0[:],
                    op0=ALU.subtract, op1=ALU.mult,
                )  # (xh*y0^2 - 1.5)*y0 = -y1
                return z, xh

            def newton_again(z, xh, n, pool=tiny):
                """One more Newton step; input negated -> output positive."""
                a = pool.tile([128, n], F32, tag="rs_a2")
                nc.vector.tensor_tensor(out=a[:], in0=z[:], in1=z[:], op=ALU.mult)
                nc.vector.tensor_tensor(out=a[:], in0=a[:], in1=xh[:], op=ALU.mult)
                y = pool.tile([128, n], F32, tag="rs_y2")
                nc.vector.scalar_tensor_tensor(
                    out=y[:], in0=a[:], scalar=1.5, in1=z[:],
                    op0=ALU.subtract, op1=ALU.mult,
                )
                return y

            def wv_dma(gl, vh):
                wvbig = wvraw.tile([128, 16, PROJ], BF16, tag="wvbig")
                nc.gpsimd.dma_start(
                    out=wvbig[:],
                    in_=wv_h.ap()[gl, vh * 2048 : (vh + 1) * 2048, :].rearrange(
                        "(a p) n -> p a n", p=128
                    ),
                )
                return wvbig

            # ---------------- startup: critical-path DMAs first ----------------
            # mm3 of block 0 first needs wv half-1 (columns 0..2047); mm1 needs
            # w1[0] + zt[0] + b1. Issue those before the rest of the constants.
            wv_first = wv_dma(0, 0)
            w1_sbs = {}
            w1_first = w1p.tile([128, KI, HID], BF16, tag="w1")
            w1_sbs[0] = w1_first
            nc.gpsimd.dma_start(out=w1_first[:, 0 : KI // 2, :], in_=w1_h.ap()[0, :, 0 : KI // 2, :])
            nc.gpsimd.dma_start(out=w1_first[:, KI // 2 :, :], in_=w1_h.ap()[0, :, KI // 2 :, :])
            zt0 = ztp.tile([128, KI, 128], BF16, tag="zt")
            nc.sync.dma_start(out=zt0[:], in_=zt_h.ap()[0])
            b1_rep = consts.tile([128, GPC, HID], BF16)
            nc.gpsimd.dma_start(out=b1_rep[:], in_=_bc(b1_h.ap()))
            wv_second = wv_dma(0, 1)

            ident = consts.tile([128, 128], BF16)
            make_identity(nc, ident[:])

            b2_rep = consts.tile([128, GPC, PROJ], F32)
            nc.gpsimd.dma_start(out=b2_rep[:], in_=_bc(b2_h.ap()))
            bv_rep = consts.tile([128, GPC, CHUNK], BF16)
            nc.gpsimd.dma_start(out=bv_rep[:], in_=_bc(bv_h.ap()))
            lngn_sb = consts.tile([128, GPC, TH], F32)
            nc.gpsimd.dma_start(out=lngn_sb[:], in_=lngn_h.ap())
            lnb_sb = consts.tile([128, GPC, TH], F32)
            nc.gpsimd.dma_start(out=lnb_sb[:], in_=lnb_h.ap())
            w2_sb = consts.tile([128, GPC, TH, PROJ], BF16)
            nc.gpsimd.dma_start(out=w2_sb[:], in_=w2_h.ap().rearrange("g p t n -> p g t n"))
            sneg_sb = consts.tile([128, GPC], F32)
            nc.gpsimd.dma_start(out=sneg_sb[:], in_=_bc(sneg_h.ap()))

            def w1_load(gl):
                t = w1p.tile([128, KI, HID], BF16, tag="w1")
                nc.gpsimd.dma_start(out=t[:], in_=w1_h.ap()[gl])
                return t

            def wv_prep_part(wT, wvbig, vh, lo, hi):
                """Row-normalize + transpose rows [lo,hi) of one Wv half."""
                n = hi - lo
                wss = wvs.tile([128, n], F32, tag="wss")
                for i in range(lo, hi):
                    wjunk = wvs.tile([128, PROJ], BF16, tag="wjunk")
                    nc.scalar.activation(
                        out=wjunk[:], in_=wvbig[:, i, :], func=AF.Square,
                        accum_out=wss[:, i - lo : i - lo + 1],
                    )
                zneg, wxh = rsqrt_neg(wss[:], n, 1e-24)
                rw = newton_again(zneg, wxh, n)  # positive rsqrt, 2 Newtons
                for i in range(lo, hi):
                    vb = vh * 16 + i
                    wn = wvs.tile([128, PROJ], BF16, tag="wn")
                    nc.vector.tensor_scalar_mul(
                        out=wn[:], in0=wvbig[:, i, :], scalar1=rw[:, i - lo : i - lo + 1]
                    )
                    for j in range(2):
                        ptw = psT.tile([128, 128], BF16, tag="pt")
                        nc.tensor.transpose(
                            out=ptw[:], in_=wn[:, j * 128 : (j + 1) * 128],
                            identity=ident[:],
                        )
                        nc.scalar.activation(
                            out=wT[:, j, vb * 128 : (vb + 1) * 128], in_=ptw[:],
                            func=AF.Copy,
                        )

            def wv_prep_half(wT, wvbig, vh):
                wv_prep_part(wT, wvbig, vh, 0, 16)

            def wv_prep(gl, halves=None):
                """Wv row-normalize + transpose -> wT [128, 2, CHUNK]."""
                wT = wtp.tile([128, 2, CHUNK], BF16, tag="wT")
                if halves is None:
                    halves = [wv_dma(gl, 0), wv_dma(gl, 1)]
                for vh in range(2):
                    wv_prep_half(wT, halves[vh], vh)
                return wT

            wTs = {0: wv_prep(0, halves=[wv_first, wv_second])}
            wv_pend = {}

            for gl in range(GPC):
                w1_sb = w1_sbs[gl]
                wT = wTs[gl]

                # ---------------- main loop over batch blocks ----------------
                for bb in range(NB):
                    # software-pipeline next group's weight loads + Wv prep
                    # into the tail of this group's block loop, spread out to
                    # avoid DMA/ACT bursts
                    if gl + 1 < GPC and bb == NB - 16:
                        w1_sbs[gl + 1] = w1_load(gl + 1)
                    if gl + 1 < GPC and bb == NB - 14:
                        wv_pend[0] = wv_dma(gl + 1, 0)
                        nwT = wtp.tile([128, 2, CHUNK], BF16, tag="wT")
                        wTs[gl + 1] = nwT
                    if gl + 1 < GPC and bb in (NB - 12, NB - 10, NB - 8, NB - 6):
                        qi = (bb - (NB - 12)) // 2
                        wv_prep_part(wTs[gl + 1], wv_pend[0], 0, qi * 4, qi * 4 + 4)
                        if bb == NB - 10:
                            wv_pend[1] = wv_dma(gl + 1, 1)
                    if gl + 1 < GPC and bb in (NB - 5, NB - 3):
                        hi2 = (bb - (NB - 5)) // 2
                        wv_prep_part(wTs[gl + 1], wv_pend[1], 1, hi2 * 8, hi2 * 8 + 8)
                    if gl == 0 and bb == 0:
                        zt_t = zt0
                    else:
                        zt_t = ztp.tile([128, KI, 128], BF16, tag="zt")
                        nc.sync.dma_start(out=zt_t[:], in_=zt_h.ap()[bb])

                    # mm1: h = z @ W1 (+b1), into 4 psum tiles of [128, 512]
                    h_sb = hp.tile([128, HID], BF16)
                    hsum = small.tile([128, 4], F32, tag="hsum")
                    hsq = small.tile([128, 4], F32, tag="hsq")
                    for nt in range(4):
                        ph = psA.tile([128, 512], F32)
                        for k in range(KI):
                            nc.tensor.matmul(
                                ph[:], zt_t[:, k, :],
                                w1_sb[:, k, nt * 512 : (nt + 1) * 512],
                                start=(k == 0), stop=(k == KI - 1),
                            )
                        hs = h_sb[:, nt * 512 : (nt + 1) * 512]
                        nc.vector.scalar_tensor_tensor(
                            out=hs, in0=ph[:], scalar=0.0,
                            in1=b1_rep[:, gl, nt * 512 : (nt + 1) * 512],
                            op0=ALU.add, op1=ALU.add,
                            accum_out=hsum[:, nt : nt + 1],
                        )
                        hjunk = small.tile([128, 512], BF16, tag="hjunk")
                        nc.scalar.activation(
                            out=hjunk[:], in_=hs, func=AF.Square,
                            accum_out=hsq[:, nt : nt + 1],
                        )

                    # mean/var from accumulated sums, -rstd via DVE rsqrt
                    hsumt = tiny.tile([128, 1], F32, tag="hsumt")
                    nc.vector.reduce_sum(hsumt[:], hsum[:], axis=mybir.AxisListType.X)
                    hsqt = tiny.tile([128, 1], F32, tag="hsqt")
                    nc.vector.reduce_sum(hsqt[:], hsq[:], axis=mybir.AxisListType.X)
                    mean = tiny.tile([128, 1], F32, tag="mean")
                    nc.vector.tensor_scalar_mul(out=mean[:], in0=hsumt[:],
                                                scalar1=1.0 / HID)
                    msq = tiny.tile([128, 1], F32, tag="msq")
                    nc.vector.tensor_tensor(out=msq[:], in0=mean[:], in1=mean[:],
                                            op=ALU.mult)
                    # hmadj = 0.5*msq - 0.5*eps
                    nc.vector.tensor_scalar(out=msq[:], in0=msq[:], scalar1=0.5,
                                            scalar2=0.5 * LN_EPS, op0=ALU.mult,
                                            op1=ALU.subtract)
                    # xh = (var+eps)/2 = hsqt*(0.5/HID) - hmadj
                    vxh = tiny.tile([128, 1], F32, tag="vxh")
                    nc.vector.scalar_tensor_tensor(
                        out=vxh[:], in0=hsqt[:], scalar=0.5 / HID, in1=msq[:],
                        op0=ALU.mult, op1=ALU.subtract,
                    )
                    # seed + one Newton (negated result)
                    y0 = tiny.tile([128, 1], F32, tag="ln_y0")
                    nc.vector.tensor_scalar(out=y0[:].bitcast(I32),
                                            in0=vxh[:].bitcast(I32), scalar1=1,
                                            scalar2=None, op0=ALU.arith_shift_right)
                    nc.vector.tensor_scalar(out=y0[:].bitcast(I32),
                                            in0=y0[:].bitcast(I32), scalar1=-1,
                                            scalar2=MAGIC, op0=ALU.mult, op1=ALU.add)
                    aa = tiny.tile([128, 1], F32, tag="ln_a")
                    nc.vector.tensor_tensor(out=aa[:], in0=y0[:], in1=y0[:], op=ALU.mult)
                    nc.vector.tensor_scalar(out=aa[:], in0=aa[:], scalar1=vxh[:],
                                            scalar2=1.5, op0=ALU.mult, op1=ALU.subtract)
                    nrstd = tiny.tile([128, 1], F32, tag="nrstd")
                    nc.vector.tensor_tensor(out=nrstd[:], in0=aa[:], in1=y0[:],
                                            op=ALU.mult)
                    # h = (h - mu) * (-rstd)  [negated; fixed by -ln_g ACT scale]
                    nc.vector.tensor_scalar(
                        out=h_sb[:], in0=h_sb[:], scalar1=mean[:], scalar2=nrstd[:],
                        op0=ALU.subtract, op1=ALU.mult,
                    )

                    # transpose + fused LN-affine (-g) + exact GELU
                    hT = htp.tile([128, TH, 128], BF16)
                    for t in range(TH):
                        pt = psT.tile([128, 128], BF16, tag="pt")
                        nc.tensor.transpose(
                            out=pt[:], in_=h_sb[:, t * 128 : (t + 1) * 128],
                            identity=ident[:],
                        )
                        nc.scalar.activation(
                            out=hT[:, t, :], in_=pt[:], func=(GELU_FUNC or AF.Gelu),
                            scale=lngn_sb[:, gl, t : t + 1],
                            bias=lnb_sb[:, gl, t : t + 1],
                        )

                    # mm2: q = h @ W2
                    pq = psQ.tile([128, PROJ], F32)
                    for t in range(TH):
                        nc.tensor.matmul(
                            pq[:], hT[:, t, :], w2_sb[:, gl, t, :],
                            start=(t == 0), stop=(t == TH - 1),
                        )
                    q_sb = small.tile([128, PROJ], F32, tag="q_sb")
                    nc.vector.tensor_tensor(
                        out=q_sb[:], in0=pq[:], in1=b2_rep[:, gl, :], op=ALU.add
                    )
                    qjunk = small.tile([128, PROJ], BF16, tag="qjunk")
                    qss = tiny.tile([128, 1], F32, tag="qss")
                    nc.scalar.activation(
                        out=qjunk[:], in_=q_sb[:], func=AF.Square, accum_out=qss[:],
                    )
                    nrq, _ = rsqrt_neg(qss[:], 1, 1e-24)
                    qsc = tiny.tile([128, 1], F32, tag="qsc")
                    nc.vector.tensor_tensor(
                        out=qsc[:], in0=nrq[:], in1=sneg_sb[:, gl : gl + 1], op=ALU.mult
                    )
                    qn = small.tile([128, PROJ], BF16, tag="qn")
                    nc.vector.tensor_scalar_mul(out=qn[:], in0=q_sb[:], scalar1=qsc[:])
                    qT = small.tile([128, 2, 128], BF16, tag="qT")
                    for j in range(2):
                        ptq = psT.tile([128, 128], BF16, tag="pt")
                        nc.tensor.transpose(
                            out=ptq[:], in_=qn[:, j * 128 : (j + 1) * 128],
                            identity=ident[:],
                        )
                        nc.scalar.activation(out=qT[:, j, :], in_=ptq[:], func=AF.Copy)

                    # mm3: logits = q @ wT (+bv), 8 tiles of 512
                    for vh in range(2):
                        lo = lop.tile([128, 4, 512], OUT_DT)
                        for v4 in range(4):
                            vt = vh * 4 + v4
                            pl = psL.tile([128, 512], F32)
                            nc.tensor.matmul(
                                pl[:], qT[:, 0, :],
                                wT[:, 0, vt * 512 : (vt + 1) * 512],
                                start=True, stop=False,
                            )
                            nc.tensor.matmul(
                                pl[:], qT[:, 1, :],
                                wT[:, 1, vt * 512 : (vt + 1) * 512],
                                start=False, stop=True,
                            )
                            nc.vector.tensor_tensor(
                                out=lo[:, v4, :], in0=pl[:],
                                in1=bv_rep[:, gl, vt * 512 : (vt + 1) * 512],
                                op=ALU.add,
                            )
                        nc.sync.dma_start(
                            out=out_h.ap()[
                                bb * 128 : (bb + 1) * 128,
                                gl * CHUNK + vh * 2048 : gl * CHUNK + (vh + 1) * 2048,
                            ],
                            in_=lo[:].rearrange("p a b -> p (a b)"),
                        )

    nc.compile()
    return nc


def _make_runner(nc):
    """Reusable jitted SPMD executor (mirrors bass2jax.run_bass_via_pjrt)."""
    import jax
    from jax.sharding import Mesh, PartitionSpec, NamedSharding
    from jax.experimental.shard_map import shard_map
    from concourse.bass2jax import _bass_exec_p, partition_id_tensor, install_neuronx_cc_hook

    install_neuronx_cc_hook()
    partition_name = nc.partition_id_tensor.name if nc.partition_id_tensor else None
    in_names, out_names, out_avals = [], [], []
    for alloc in nc.m.functions[0].allocations:
        if not isinstance(alloc, mybir.MemoryLocationSet):
            continue
        name = alloc.memorylocations[0].name
        if alloc.kind == "ExternalInput":
            if name != partition_name:
                in_names.append(name)
        elif alloc.kind == "ExternalOutput":
            out_names.append(name)
            out_avals.append(
                jax.core.ShapedArray(tuple(alloc.tensor_shape), mybir.dt.np(alloc.dtype))
            )
    n_params = len(in_names)
    all_in_names = in_names + out_names
    if partition_name is not None:
        all_in_names.append(partition_name)

    def _body(*args):
        operands = list(args)
        if partition_name is not None:
            operands.append(partition_id_tensor())
        return tuple(
            _bass_exec_p.bind(
                *operands,
                out_avals=tuple(out_avals),
                in_names=tuple(all_in_names),
                out_names=tuple(out_names),
                lowering_input_output_aliases=(),
                sim_require_finite=True,
                sim_require_nnan=True,
                nc=nc,
            )
        )

    devices = jax.devices()[:NCORES]
    mesh = Mesh(np.asarray(devices), ("core",))
    spec = NamedSharding(mesh, PartitionSpec("core"))
    n_out = len(out_names)
    fn = jax.jit(
        shard_map(
            _body, mesh=mesh,
            in_specs=(PartitionSpec("core"),) * (n_params + n_out),
            out_specs=(PartitionSpec("core"),) * n_out,
            check_rep=False,
        ),
        keep_unused=True,
    )

    def put(in_maps):
        import jax as _jax
        concat = [
            _jax.device_put(
                np.concatenate([np.asarray(in_maps[c][nm]) for c in range(NCORES)], axis=0),
                spec,
            )
            for nm in in_names
        ]
        zeros = [
            _jax.device_put(
                np.zeros((NCORES * a.shape[0], *a.shape[1:]), a.dtype), spec
            )
            for a in out_avals
        ]
        return concat + zeros

    def run(args):
        outs = fn(*args)
        return outs, out_names, out_avals

    return put, run


def _prep_inputs(z, W1, b1, ln_g, ln_b, W2, b2, Wv, bv, logit_scale):
    bf = ml_dtypes.bfloat16
    zt = np.ascontiguousarray(
        z.T.reshape(KI, 128, NB, 128).transpose(2, 1, 0, 3)
    ).astype(bf)  # [bb, p, k, b]
    s = np.minimum(np.exp(logit_scale.astype(np.float64)), 100.0).astype(np.float32)
    in_maps = []
    for c in range(NCORES):
        gs = slice(GPC * c, GPC * (c + 1))
        w1c = np.ascontiguousarray(
            W1[gs].reshape(GPC, KI, 128, HID).transpose(0, 2, 1, 3)
        ).astype(bf)  # [g, p, k, n]
        w2c = np.ascontiguousarray(
            W2[gs].reshape(GPC, TH, 128, PROJ).transpose(0, 2, 1, 3)
        ).astype(bf)  # [g, p, t, n]
        lngnc = np.ascontiguousarray(
            (-ln_g[gs]).reshape(GPC, TH, 128).transpose(2, 0, 1)
        ).astype(np.float32)  # [p, g, t], negated
        lnbc = np.ascontiguousarray(
            ln_b[gs].reshape(GPC, TH, 128).transpose(2, 0, 1)
        ).astype(np.float32)
        in_maps.append(
            {
                "zt": zt,
                "w1": w1c,
                "w2": w2c,
                "wv": Wv[gs].astype(bf),
                "b1": b1[gs].astype(bf),
                "b2": b2[gs].astype(np.float32),
                "bv": bv[gs].astype(bf),
                "lngn": lngnc,
                "lnb": lnbc,
                "sneg": -s[gs],
            }
        )
    return in_maps


def _get_runtime():
    global _RT
    if _RT is None:
        nc = _build()
        put, run = _make_runner(nc)
        _RT = (nc, put, run)
    return _RT


def kernel(**inputs):
    inputs = {k: np.asarray(v) for k, v in inputs.items()}
    in_maps = _prep_inputs(**inputs)
    _, put, run = _get_runtime()
    args = put(in_maps)
    outs, out_names, out_avals = run(args)
    out = np.asarray(outs[out_names.index("out")])
    out = out.reshape(NCORES, B, GPC * CHUNK)
    return np.concatenate(list(out), axis=1).astype(np.float32)


# revision 9
# speedup vs baseline: 1.1337x; 1.0085x over previous
"""Grouped projected head on 8 TRN2 NeuronCores — v2.

Sharding: group axis G=16 split across 8 cores (2 groups/core, expert-parallel).
z replicated (pre-transposed + bf16 on host). Per core, for its groups g:
    h = z @ W1[g] + b1[g] -> LayerNorm -> GELU(exact)
    q = h @ W2[g] + b2[g] -> L2 normalize -> * min(exp(ls[g]),100)
    logits = q @ normalize(Wv[g]).T + bv[g]

v2 changes vs baseline:
  - No ACT table thrash: every ACT func in steady state is in the
    gelu_and_others set (Gelu, Square, Copy) -> one table load total.
    sqrt/rsqrt done on DVE via bit-trick seed + one Newton step; the result
    is produced NEGATED and the sign is absorbed into host-negated
    constants (-ln_g for the LN scale, -min(exp(ls),100) for the q scale)
    or an extra *-1 ALU stage (Wv rows).
  - LN stats: h-evacuation is scalar_tensor_tensor with accum_out (sum h
    for free); sum h^2 via ACT Square+accum (idle engine) -> bn_stats and
    its DVE cost removed.
  - PSUM->SBUF transpose-evacuation copies moved from DVE to ACT.
  - min(exp(logit_scale),100) precomputed on host.
  - Optional bf16 DRAM output (halves output HBM traffic), upcast on host.
"""

import sys

sys.path.insert(0, "/opt/trn_rl_repo")

import numpy as np
import ml_dtypes

import concourse.bass as bass
from concourse import bacc, mybir, tile

BF16 = mybir.dt.bfloat16
F32 = mybir.dt.float32
I32 = mybir.dt.int32
AF = mybir.ActivationFunctionType
ALU = mybir.AluOpType

B, G, IN, HID, PROJ, CHUNK = 4096, 16, 1024, 2048, 256, 4096
NCORES = 8
GPC = G // NCORES          # groups per core
NB = B // 128              # 32 batch blocks
KI = IN // 128             # 8 k-chunks for mm1
TH = HID // 128            # 16 hid-chunks
NVB = CHUNK // 128         # 32 Wv row blocks
LN_EPS = 1e-5
MAGIC = 0x5EF759DF         # rsqrt seed magic for half-x input
GELU_FUNC = None           # set to AF.Tanh for sim debug
OUT_BF16 = True
OUT_DT = BF16 if OUT_BF16 else F32

_RT = None  # cached (nc, put, run)


def _bc(ap, parts=128):
    """Partition-broadcast a DRAM AP (stride-0 partition dim) for DMA."""
    return bass.AP(tensor=ap.tensor, offset=ap.offset, ap=[[0, parts], *ap.ap])


def _build():
    nc = bacc.Bacc("TRN2", target_bir_lowering=False, debug=False)

    zt_h = nc.dram_tensor("zt", [NB, 128, KI, 128], BF16, kind="ExternalInput")
    w1_h = nc.dram_tensor("w1", [GPC, 128, KI, HID], BF16, kind="ExternalInput")
    w2_h = nc.dram_tensor("w2", [GPC, 128, TH, PROJ], BF16, kind="ExternalInput")
    wv_h = nc.dram_tensor("wv", [GPC, CHUNK, PROJ], BF16, kind="ExternalInput")
    b1_h = nc.dram_tensor("b1", [GPC, HID], BF16, kind="ExternalInput")
    b2_h = nc.dram_tensor("b2", [GPC, PROJ], F32, kind="ExternalInput")
    bv_h = nc.dram_tensor("bv", [GPC, CHUNK], BF16, kind="ExternalInput")
    lngn_h = nc.dram_tensor("lngn", [128, GPC, TH], F32, kind="ExternalInput")
    lnb_h = nc.dram_tensor("lnb", [128, GPC, TH], F32, kind="ExternalInput")
    sneg_h = nc.dram_tensor("sneg", [GPC], F32, kind="ExternalInput")
    out_h = nc.dram_tensor("out", [B, GPC * CHUNK], OUT_DT, kind="ExternalOutput")

    with tile.TileContext(nc) as tc:
        with (
            tc.tile_pool(name="consts", bufs=1) as consts,
            tc.tile_pool(name="w1p", bufs=2) as w1p,
            tc.tile_pool(name="wtp", bufs=2) as wtp,
            tc.tile_pool(name="ztp", bufs=4) as ztp,
            tc.tile_pool(name="hp", bufs=3) as hp,
            tc.tile_pool(name="htp", bufs=2) as htp,
            tc.tile_pool(name="small", bufs=3) as small,
            tc.tile_pool(name="tiny", bufs=4) as tiny,
            tc.tile_pool(name="wvraw", bufs=2) as wvraw,
            tc.tile_pool(name="wvs", bufs=2) as wvs,
            tc.tile_pool(name="lop", bufs=2) as lop,
            tc.tile_pool(name="psA", bufs=2, space="PSUM") as psA,
            tc.tile_pool(name="psT", bufs=3, space="PSUM") as psT,
            tc.tile_pool(name="psQ", bufs=1, space="PSUM") as psQ,
            tc.tile_pool(name="psL", bufs=2, space="PSUM") as psL,
        ):
            from concourse.masks import make_identity

            def rsqrt_neg(x_ap, n, eps, pool=tiny):
                """Emit DVE ops computing ~ -1/sqrt(x+eps) for [128, n] fp32.

                Bit-trick seed + one Newton step; returns the tile holding the
                NEGATED result (|rel err| <= ~1.8e-3)."""
                xh = pool.tile([128, n], F32, tag="rs_xh")
                nc.vector.tensor_scalar(out=xh[:], in0=x_ap, scalar1=eps,
                                        scalar2=0.5, op0=ALU.add, op1=ALU.mult)
                y0 = pool.tile([128, n], F32, tag="rs_y0")
                nc.vector.tensor_scalar(out=y0[:].bitcast(I32),
                                        in0=xh[:].bitcast(I32), scalar1=1,
                                        scalar2=None, op0=ALU.arith_shift_right)
                nc.vector.tensor_scalar(out=y0[:].bitcast(I32),
                                        in0=y0[:].bitcast(I32), scalar1=-1,
                                        scalar2=MAGIC, op0=ALU.mult, op1=ALU.add)
                a = pool.tile([128, n], F32, tag="rs_a")
                nc.vector.tensor_tensor(out=a[:], in0=y0[:], in1=y0[:], op=ALU.mult)
                nc.vector.tensor_tensor(out=a[:], in0=a[:], in1=xh[:], op=ALU.mult)
                z = pool.tile([128, n], F32, tag="rs_z")
                nc.vector.scalar_tensor_tensor(
                    out=z[:], in0=a[:], scalar=1.5, in1=y0[:],
                    op0=ALU.subtract, op1=ALU.mult,
                )  # (xh*y0^2 - 1.5)*y0 = -y1
                return z, xh

            def newton_again(z, xh, n, pool=tiny):
                """One more Newton step; input negated -> output positive."""
                a = pool.tile([128, n], F32, tag="rs_a2")
                nc.vector.tensor_tensor(out=a[:], in0=z[:], in1=z[:], op=ALU.mult)
                nc.vector.tensor_tensor(out=a[:], in0=a[:], in1=xh[:], op=ALU.mult)
                y = pool.tile([128, n], F32, tag="rs_y2")
                nc.vector.scalar_tensor_tensor(
                    out=y[:], in0=a[:], scalar=1.5, in1=z[:],
                    op0=ALU.subtract, op1=ALU.mult,
                )
                return y

            def wv_dma(gl, vh):
                wvbig = wvraw.tile([128, 16, PROJ], BF16, tag="wvbig")
                nc.gpsimd.dma_start(
                    out=wvbig[:],
                    in_=wv_h.ap()[gl, vh * 2048 : (vh + 1) * 2048, :].rearrange(
                        "(a p) n -> p a n", p=128
                    ),
                )
                return wvbig

            # ---------------- startup: critical-path DMAs first ----------------
            # mm3 of block 0 first needs wv half-1 (columns 0..2047); mm1 needs
            # w1[0] + zt[0] + b1. Issue those before the rest of the constants.
            wv_first = wv_dma(0, 0)
            w1_sbs = {}
            w1_first = w1p.tile([128, KI, HID], BF16, tag="w1")
            w1_sbs[0] = w1_first
            nc.gpsimd.dma_start(out=w1_first[:, 0 : KI // 2, :], in_=w1_h.ap()[0, :, 0 : KI // 2, :])
            nc.gpsimd.dma_start(out=w1_first[:, KI // 2 :, :], in_=w1_h.ap()[0, :, KI // 2 :, :])
            zt0 = ztp.tile([128, KI, 128], BF16, tag="zt")
            nc.sync.dma_start(out=zt0[:], in_=zt_h.ap()[0])
            b1_rep = consts.tile([128, GPC, HID], BF16)
            nc.gpsimd.dma_start(out=b1_rep[:], in_=_bc(b1_h.ap()))
            wv_second = wv_dma(0, 1)

            ident = consts.tile([128, 128], BF16)
            make_identity(nc, ident[:])

            b2_rep = consts.tile([128, GPC, PROJ], F32)
            nc.gpsimd.dma_start(out=b2_rep[:], in_=_bc(b2_h.ap()))
            bv_rep = consts.tile([128, GPC, CHUNK], BF16)
            nc.gpsimd.dma_start(out=bv_rep[:], in_=_bc(bv_h.ap()))
            lngn_sb = consts.tile([128, GPC, TH], F32)
            nc.gpsimd.dma_start(out=lngn_sb[:], in_=lngn_h.ap())
            lnb_sb = consts.tile([128, GPC, TH], F32)
            nc.gpsimd.dma_start(out=lnb_sb[:], in_=lnb_h.ap())
            w2_sb = consts.tile([128, GPC, TH, PROJ], BF16)
            nc.gpsimd.dma_start(out=w2_sb[:], in_=w2_h.ap().rearrange("g p t n -> p g t n"))
            sneg_sb = consts.tile([128, GPC], F32)
            nc.gpsimd.dma_start(out=sneg_sb[:], in_=_bc(sneg_h.ap()))

            def w1_load(gl):
                t = w1p.tile([128, KI, HID], BF16, tag="w1")
                nc.gpsimd.dma_start(out=t[:], in_=w1_h.ap()[gl])
                return t

            def wv_prep_part(wT, wvbig, vh, lo, hi):
                """Row-normalize + transpose rows [lo,hi) of one Wv half."""
                n = hi - lo
                wss = wvs.tile([128, n], F32, tag="wss")
                for i in range(lo, hi):
                    wjunk = wvs.tile([128, PROJ], BF16, tag="wjunk")
                    nc.scalar.activation(
                        out=wjunk[:], in_=wvbig[:, i, :], func=AF.Square,
                        accum_out=wss[:, i - lo : i - lo + 1],
                    )
                zneg, wxh = rsqrt_neg(wss[:], n, 1e-24)
                rw = newton_again(zneg, wxh, n)  # positive rsqrt, 2 Newtons
                for i in range(lo, hi):
                    vb = vh * 16 + i
                    wn = wvs.tile([128, PROJ], BF16, tag="wn")
                    nc.vector.tensor_scalar_mul(
                        out=wn[:], in0=wvbig[:, i, :], scalar1=rw[:, i - lo : i - lo + 1]
                    )
                    for j in range(2):
                        ptw = psT.tile([128, 128], BF16, tag="pt")
                        nc.tensor.transpose(
                            out=ptw[:], in_=wn[:, j * 128 : (j + 1) * 128],
                            identity=ident[:],
                        )
                        nc.scalar.activation(
                            out=wT[:, j, vb * 128 : (vb + 1) * 128], in_=ptw[:],
                            func=AF.Copy,
                        )

            def wv_prep_half(wT, wvbig, vh):
                wv_prep_part(wT, wvbig, vh, 0, 16)

            def wv_prep(gl, halves=None):
                """Wv row-normalize + transpose -> wT [128, 2, CHUNK]."""
                wT = wtp.tile([128, 2, CHUNK], BF16, tag="wT")
                if halves is None:
                    halves = [wv_dma(gl, 0), wv_dma(gl, 1)]
                for vh in range(2):
                    wv_prep_half(wT, halves[vh], vh)
                return wT

            wTs = {0: wv_prep(0, halves=[wv_first, wv_second])}
            wv_pend = {}

            for gl in range(GPC):
                w1_sb = w1_sbs[gl]
                wT = wTs[gl]

                # ---------------- main loop over batch blocks ----------------
                for bb in range(NB):
                    # software-pipeline next group's weight loads + Wv prep
                    # into the tail of this group's block loop, spread out to
                    # avoid DMA/ACT bursts
                    if gl + 1 < GPC and bb == NB - 16:
                        w1_sbs[gl + 1] = w1_load(gl + 1)
                    if gl + 1 < GPC and bb == NB - 14:
                        wv_pend[0] = wv_dma(gl + 1, 0)
                        nwT = wtp.tile([128, 2, CHUNK], BF16, tag="wT")
                        wTs[gl + 1] = nwT
                    if gl + 1 < GPC and bb in (NB - 12, NB - 10, NB - 8, NB - 6):
                        qi = (bb - (NB - 12)) // 2
                        wv_prep_part(wTs[gl + 1], wv_pend[0], 0, qi * 4, qi * 4 + 4)
                        if bb == NB - 10:
                            wv_pend[1] = wv_dma(gl + 1, 1)
                    if gl + 1 < GPC and bb in (NB - 5, NB - 3):
                        hi2 = (bb - (NB - 5)) // 2
                        wv_prep_part(wTs[gl + 1], wv_pend[1], 1, hi2 * 8, hi2 * 8 + 8)
                    if gl == 0 and bb == 0:
                        zt_t = zt0
                    else:
                        zt_t = ztp.tile([128, KI, 128], BF16, tag="zt")
                        nc.sync.dma_start(out=zt_t[:], in_=zt_h.ap()[bb])

                    # mm1: h = z @ W1 (+b1), into 4 psum tiles of [128, 512]
                    h_sb = hp.tile([128, HID], BF16)
                    hsum = small.tile([128, 4], F32, tag="hsum")
                    hsq = small.tile([128, 4], F32, tag="hsq")
                    for nt in range(4):
                        ph = psA.tile([128, 512], F32)
                        for k in range(KI):
                            nc.tensor.matmul(
                                ph[:], zt_t[:, k, :],
                                w1_sb[:, k, nt * 512 : (nt + 1) * 512],
                                start=(k == 0), stop=(k == KI - 1),
                            )
                        hs = h_sb[:, nt * 512 : (nt + 1) * 512]
                        nc.vector.scalar_tensor_tensor(
                            out=hs, in0=ph[:], scalar=0.0,
                            in1=b1_rep[:, gl, nt * 512 : (nt + 1) * 512],
                            op0=ALU.add, op1=ALU.add,
                            accum_out=hsum[:, nt : nt + 1],
                        )
                        hjunk = small.tile([128, 512], BF16, tag="hjunk")
                        nc.scalar.activation(
                            out=hjunk[:], in_=hs, func=AF.Square,
                            accum_out=hsq[:, nt : nt + 1],
                        )

                    # mean/var from accumulated sums, -rstd via DVE rsqrt
                    hsumt = tiny.tile([128, 1], F32, tag="hsumt")
                    nc.vector.reduce_sum(hsumt[:], hsum[:], axis=mybir.AxisListType.X)
                    hsqt = tiny.tile([128, 1], F32, tag="hsqt")
                    nc.vector.reduce_sum(hsqt[:], hsq[:], axis=mybir.AxisListType.X)
                    mean = tiny.tile([128, 1], F32, tag="mean")
                    nc.vector.tensor_scalar_mul(out=mean[:], in0=hsumt[:],
                                                scalar1=1.0 / HID)
                    msq = tiny.tile([128, 1], F32, tag="msq")
                    nc.vector.tensor_tensor(out=msq[:], in0=mean[:], in1=mean[:],
                                            op=ALU.mult)
                    # hmadj = 0.5*msq - 0.5*eps
                    nc.vector.tensor_scalar(out=msq[:], in0=msq[:], scalar1=0.5,
                                            scalar2=0.5 * LN_EPS, op0=ALU.mult,
                                            op1=ALU.subtract)
                    # xh = (var+eps)/2 = hsqt*(0.5/HID) - hmadj
                    vxh = tiny.tile([128, 1], F32, tag="vxh")
                    nc.vector.scalar_tensor_tensor(
                        out=vxh[:], in0=hsqt[:], scalar=0.5 / HID, in1=msq[:],
                        op0=ALU.mult, op1=ALU.subtract,
                    )
                    # seed + one Newton (negated result)
                    y0 = tiny.tile([128, 1], F32, tag="ln_y0")
                    nc.vector.tensor_scalar(out=y0[:].bitcast(I32),
                                            in0=vxh[:].bitcast(I32), scalar1=1,
                                            scalar2=None, op0=ALU.arith_shift_right)
                    nc.vector.tensor_scalar(out=y0[:].bitcast(I32),
                                            in0=y0[:].bitcast(I32), scalar1=-1,
                                            scalar2=MAGIC, op0=ALU.mult, op1=ALU.add)
                    aa = tiny.tile([128, 1], F32, tag="ln_a")
                    nc.vector.tensor_tensor(out=aa[:], in0=y0[:], in1=y0[:], op=ALU.mult)
                    nc.vector.tensor_scalar(out=aa[:], in0=aa[:], scalar1=vxh[:],
                                            scalar2=1.5, op0=ALU.mult, op1=ALU.subtract)
                    nrstd = tiny.tile([128, 1], F32, tag="nrstd")
                    nc.vector.tensor_tensor(out=nrstd[:], in0=aa[:], in1=y0[:],
                                            op=ALU.mult)
                    # h = (h - mu) * (-rstd)  [negated; fixed by -ln_g ACT scale]
                    nc.vector.tensor_scalar(
                        out=h_sb[:], in0=h_sb[:], scalar1=mean[:], scalar2=nrstd[:],
                        op0=ALU.subtract, op1=ALU.mult,
                    )

                    # transpose + fused LN-affine (-g) + exact GELU
                    hT = htp.tile([128, TH, 128], BF16)
                    for t in range(TH):
                        pt = psT.tile([128, 128], BF16, tag="pt")
                        nc.tensor.transpose(
                            out=pt[:], in_=h_sb[:, t * 128 : (t + 1) * 128],
                            identity=ident[:],
                        )
                        nc.scalar.activation(
                            out=hT[:, t, :], in_=pt[:], func=(GELU_FUNC or AF.Gelu),
                            scale=lngn_sb[:, gl, t : t + 1],
                            bias=lnb_sb[:, gl, t : t + 1],
                        )

                    # mm2: q = h @ W2
                    pq = psQ.tile([128, PROJ], F32)
                    for t in range(TH):
                        nc.tensor.matmul(
                            pq[:], hT[:, t, :], w2_sb[:, gl, t, :],
                            start=(t == 0), stop=(t == TH - 1),
                        )
                    q_sb = small.tile([128, PROJ], F32, tag="q_sb")
                    nc.vector.tensor_tensor(
                        out=q_sb[:], in0=pq[:], in1=b2_rep[:, gl, :], op=ALU.add
                    )
                    qjunk = small.tile([128, PROJ], BF16, tag="qjunk")
                    qss = tiny.tile([128, 1], F32, tag="qss")
                    nc.scalar.activation(
                        out=qjunk[:], in_=q_sb[:], func=AF.Square, accum_out=qss[:],
                    )
                    nrq, _ = rsqrt_neg(qss[:], 1, 1e-24)
                    qsc = tiny.tile([128, 1], F32, tag="qsc")
                    nc.vector.tensor_tensor(
                        out=qsc[:], in0=nrq[:], in1=sneg_sb[:, gl : gl + 1], op=ALU.mult
                    )
                    qn = small.tile([128, PROJ], BF16, tag="qn")
                    nc.vector.tensor_scalar_mul(out=qn[:], in0=q_sb[:], scalar1=qsc[:])
                    qT = small.tile([128, 2, 128], BF16, tag="qT")
                    for j in range(2):
                        ptq = psT.tile([128, 128], BF16, tag="pt")
                        nc.tensor.transpose(
                            out=ptq[:], in_=qn[:, j * 128 : (j + 1) * 128],
                            identity=ident[:],
                        )
                        nc.scalar.activation(out=qT[:, j, :], in_=ptq[:], func=AF.Copy)

                    # mm3: logits = q @ wT (+bv), 8 tiles of 512
                    for vh in range(2):
                        lo = lop.tile([128, 4, 512], OUT_DT)
                        for v4 in range(4):
                            vt = vh * 4 + v4
                            pl = psL.tile([128, 512], F32)
                            nc.tensor.matmul(
                                pl[:], qT[:, 0, :],
                                wT[:, 0, vt * 512 : (vt + 1) * 512],
                                start=True, stop=False,
                            )
                            nc.tensor.matmul(
                                pl[:], qT[:, 1, :],
                                wT[:, 1, vt * 512 : (vt + 1) * 512],
                                start=False, stop=True,
                            )
                            nc.vector.tensor_tensor(
                                out=lo[:, v4, :], in0=pl[:],
                                in1=bv_rep[:, gl, vt * 512 : (vt + 1) * 512],
                                op=ALU.add,
                            )
                        nc.sync.dma_start(
                            out=out_h.ap()[
                                bb * 128 : (bb + 1) * 128,
                                gl * CHUNK + vh * 2048 : gl * CHUNK + (vh + 1) * 2048,
                            ],
                            in_=lo[:].rearrange("p a b -> p (a b)"),
                        )

    nc.compile()
    return nc


def _make_runner(nc):
    """Reusable jitted SPMD executor (mirrors bass2jax.run_bass_via_pjrt)."""
    import jax
    from jax.sharding import Mesh, PartitionSpec, NamedSharding
    from jax.experimental.shard_map import shard_map
    from concourse.bass2jax import _bass_exec_p, partition_id_tensor, install_neuronx_cc_hook

    install_neuronx_cc_hook()
    partition_name = nc.partition_id_tensor.name if nc.partition_id_tensor else None
    in_names, out_names, out_avals = [], [], []
    for alloc in nc.m.functions[0].allocations:
        if not isinstance(alloc, mybir.MemoryLocationSet):
            continue
        name = alloc.memorylocations[0].name
        if alloc.kind == "ExternalInput":
            if name != partition_name:
                in_names.append(name)
        elif alloc.kind == "ExternalOutput":
            out_names.append(name)
            out_avals.append(
                jax.core.ShapedArray(tuple(alloc.tensor_shape), mybir.dt.np(alloc.dtype))
            )
    n_params = len(in_names)
    all_in_names = in_names + out_names
    if partition_name is not None:
        all_in_names.append(partition_name)

    def _body(*args):
        operands = list(args)
        if partition_name is not None:
            operands.append(partition_id_tensor())
        return tuple(
            _bass_exec_p.bind(
                *operands,
                out_avals=tuple(out_avals),
                in_names=tuple(all_in_names),
                out_names=tuple(out_names),
                lowering_input_output_aliases=(),
                sim_require_finite=True,
                sim_require_nnan=True,
                nc=nc,
            )
        )

    devices = jax.devices()[:NCORES]
    mesh = Mesh(np.asarray(devices), ("core",))
    spec = NamedSharding(mesh, PartitionSpec("core"))
    n_out = len(out_names)
    fn = jax.jit(
        shard_map(
            _body, mesh=mesh,
            in_specs=(PartitionSpec("core"),) * (n_params + n_out),
            out_specs=(PartitionSpec("core"),) * n_out,
            check_rep=False,
        ),
        keep_unused=True,
    )

    def put(in_maps):
        import jax as _jax
        concat = [
            _jax.device_put(
                np.concatenate([np.asarray(in_maps[c][nm]) for c in range(NCORES)], axis=0),
                spec,
            )
            for nm in in_names
        ]
        zeros = [
            _jax.device_put(
                np.zeros((NCORES * a.shape[0], *a.shape[1:]), a.dtype), spec
            )
            for a in out_avals
        ]
        return concat + zeros

    def run(args):
        outs = fn(*args)
        return outs, out_names, out_avals

    return put, run


def _prep_inputs(z, W1, b1, ln_g, ln_b, W2, b2, Wv, bv, logit_scale):
    bf = ml_dtypes.bfloat16
    zt = np.ascontiguousarray(
        z.T.reshape(KI, 128, NB, 128).transpose(2, 1, 0, 3)
    ).astype(bf)  # [bb, p, k, b]
    s = np.minimum(np.exp(logit_scale.astype(np.float64)), 100.0).astype(np.float32)
    in_maps = []
    for c in range(NCORES):
        gs = slice(GPC * c, GPC * (c + 1))
        w1c = np.ascontiguousarray(
            W1[gs].reshape(GPC, KI, 128, HID).transpose(0, 2, 1, 3)
        ).astype(bf)  # [g, p, k, n]
        w2c = np.ascontiguousarray(
            W2[gs].reshape(GPC, TH, 128, PROJ).transpose(0, 2, 1, 3)
        ).astype(bf)  # [g, p, t, n]
        lngnc = np.ascontiguousarray(
            (-ln_g[gs]).reshape(GPC, TH, 128).transpose(2, 0, 1)
        ).astype(np.float32)  # [p, g, t], negated
        lnbc = np.ascontiguousarray(
            ln_b[gs].reshape(GPC, TH, 128).transpose(2, 0, 1)
        ).astype(np.float32)
        in_maps.append(
            {
                "zt": zt,
                "w1": w1c,
                "w2": w2c,
                "wv": Wv[gs].astype(bf),
                "b1": b1[gs].astype(bf),
                "b2": b2[gs].astype(np.float32),
                "bv": bv[gs].astype(bf),
                "lngn": lngnc,
                "lnb": lnbc,
                "sneg": -s[gs],
            }
        )
    return in_maps


def _get_runtime():
    global _RT
    if _RT is None:
        nc = _build()
        put, run = _make_runner(nc)
        _RT = (nc, put, run)
    return _RT


def kernel(**inputs):
    inputs = {k: np.asarray(v) for k, v in inputs.items()}
    in_maps = _prep_inputs(**inputs)
    _, put, run = _get_runtime()
    args = put(in_maps)
    outs, out_names, out_avals = run(args)
    out = np.asarray(outs[out_names.index("out")])
    out = out.reshape(NCORES, B, GPC * CHUNK)
    return np.concatenate(list(out), axis=1).astype(np.float32)
